# revision 13
# baseline (speedup 1.0000x reference)
"""Trainium2 Bass kernel for nn_Decoder_Block (gnn_message_passing).

Sharding: data-parallel over batch b (16 batches / 8 cores = 2 per core);
the protein path is replicated on every core.

Per-core layout strategy: the big (b,m,m,c) "ma" tensor is processed in a
transposed-primary layout — per (b, m1) the slab [m2=128, c=256] lives as
[c (2 partition halves) x m2 free].  In that layout the softmax over m2 is a
free-axis reduction, q/bias/LN-gamma are per-partition scalars, and every
matmul consumes activations directly as the moving operand (out = W^T @ Xt),
so no activation ever needs an extra transpose for a matmul.  LayerNorms
over c are computed in natural layout around the PE transposes that the
matmuls need anyway.  Matmuls run in float32r (~TF32 precision, 1 cyc/row
for moving dim >= 256); everything else is fp32.
"""

import sys

sys.path.insert(0, "/opt/trn_rl_repo")

import numpy as np

import bass_rust
import concourse.bass as bass
import concourse.tile as tile
from concourse import mybir
from concourse.masks import make_identity

F32 = mybir.dt.float32
F32R = mybir.dt.float32r
AF = mybir.ActivationFunctionType
OP = mybir.AluOpType

DIM = 256
H = 128  # half of DIM, also m2/n tile size
B, M, N = 16, 128, 128
N_CORES = 8
B_PER_CORE = B // N_CORES
EPS = 1e-5

# ---------------------------------------------------------------------------
# walrus workaround: this container's walrus rejects >1 sync wait per
# instruction; split excess waits onto NOPs inserted before the instruction.
# ---------------------------------------------------------------------------


def _split_sync_waits(nc, maxw=1):
    ctr = 0
    for fn in nc.m.functions:
        for bb in fn.blocks:
            out = []
            dirty = False
            for inst in bb.instructions:
                si = inst.sync_info
                if si is not None and len(si.on_wait) > maxw:
                    waits = list(si.on_wait)
                    rest, keep = waits[:-maxw], waits[-maxw:]
                    for i in range(0, len(rest), maxw):
                        ctr += 1
                        nop = mybir.InstNoOp(name=f"WSPL-{ctr}", ins=[], outs=[])
                        nop.engine = inst.engine
                        nop.sync_info = bass_rust.SyncInfo(
                            on_wait=rest[i : i + maxw], on_update=[]
                        )
                        out.append(nop)
                    si.on_wait = keep
                    dirty = True
                out.append(inst)
            if dirty:
                bb.instructions = out
    return ctr


# ---------------------------------------------------------------------------
# program builder
# ---------------------------------------------------------------------------

# weights consumed as lhsT blocks  W[k*128:(k+1)*128, m*128:(m+1)*128]
W_T4 = [
    "v_ma", "out_ed", "out_nd", "q_mx", "v_mx", "k_px",
    "p_q", "p_k", "p_v", "ma_fc1", "ma_fc2", "mx_fc1", "mx_fc2",
]
# weights consumed as rhs blocks  W[k*128:(k+1)*128, :]
W_R3 = ["p_out"]
# biases loaded as [128, 2] columns (transposed-layout per-partition)
B_COL = [
    "v_ma", "out_ed", "out_nd", "q_mx", "v_mx", "k_px",
    "p_q", "p_k", "p_v", "ma_fc1", "ma_fc2", "mx_fc1", "mx_fc2",
]
# biases loaded as [128, 256] partition-broadcast tiles (natural layout)
B_BC = ["p_out"]
# LN params as [128, 2] columns (used in transposed-layout ACT copies)
LN_COL = ["ln1_ma", "ln3_ma", "ln3_mx"]
# LN params as [128, 256] broadcast tiles (applied in natural layout)
LN_BC = ["ln1_mx", "ln1_px", "ln2_px", "ln4_ma", "ln4_mx"]


def build_program(split_waits=True):
    nc = bass.Bass(target_bir_lowering=False)

    ma_in = nc.dram_tensor("ma_in", [B_PER_CORE, M, M, DIM], F32, kind="ExternalInput")
    mx_in = nc.dram_tensor("mx_in", [B_PER_CORE, M, DIM], F32, kind="ExternalInput")
    pe_in = nc.dram_tensor("pe_in", [N, DIM], F32, kind="ExternalInput")

    wt = {n: nc.dram_tensor(f"w_{n}", [DIM, DIM], F32R, kind="ExternalInput")
          for n in W_T4 + W_R3}
    bt = {n: nc.dram_tensor(f"b_{n}", [DIM], F32, kind="ExternalInput")
          for n in set(B_COL + B_BC)}
    lng = {n: nc.dram_tensor(f"g_{n}", [DIM], F32, kind="ExternalInput")
           for n in LN_COL + LN_BC}
    lnb = {n: nc.dram_tensor(f"bb_{n}", [DIM], F32, kind="ExternalInput")
           for n in LN_COL + LN_BC}

    ma_out = nc.dram_tensor("ma_out", [B_PER_CORE, M, M, DIM], F32,
                            kind="ExternalOutput")
    mx_out = nc.dram_tensor("mx_out", [B_PER_CORE, M, DIM], F32,
                            kind="ExternalOutput")

    with tile.TileContext(nc) as tc:
        _emit(nc, tc, ma_in, mx_in, pe_in, wt, bt, lng, lnb, ma_out, mx_out)
    if split_waits:
        _split_sync_waits(nc)
    return nc


def _emit(nc, tc, ma_in, mx_in, pe_in, wt, bt, lng, lnb, ma_out, mx_out):
    from contextlib import ExitStack

    ctx = ExitStack()
    with ctx:
        const = ctx.enter_context(tc.tile_pool(name="const", bufs=1))
        glob = ctx.enter_context(tc.tile_pool(name="glob", bufs=1))
        work = ctx.enter_context(tc.tile_pool(name="work", bufs=2))
        work3 = ctx.enter_context(tc.tile_pool(name="work3", bufs=3))
        ps_t = ctx.enter_context(tc.tile_pool(name="ps_t", bufs=2, space="PSUM"))
        ps_mm = ctx.enter_context(tc.tile_pool(name="ps_mm", bufs=4, space="PSUM"))
        ps_nat = ctx.enter_context(tc.tile_pool(name="ps_nat", bufs=2, space="PSUM"))

        # ---------------- constants ----------------
        w = {}
        for n in W_T4:
            t = const.tile([H, 2, 2, H], F32R, tag=f"w_{n}")
            nc.sync.dma_start(
                out=t, in_=wt[n].rearrange("(k p) (m j) -> p k m j", p=H, j=H))
            w[n] = t
        for n in W_R3:
            t = const.tile([H, 2, DIM], F32R, tag=f"w_{n}")
            nc.sync.dma_start(out=t, in_=wt[n].rearrange("(k p) n -> p k n", p=H))
            w[n] = t

        bc = {}  # bias columns [128, 2]
        for n in B_COL:
            t = const.tile([H, 2], F32, tag=f"bc_{n}")
            nc.sync.dma_start(out=t, in_=bt[n].rearrange("(h p) -> p h", p=H))
            bc[n] = t
        bbc = {}  # bias broadcast [128, 256]
        for n in B_BC:
            t = const.tile([H, DIM], F32, tag=f"bbc_{n}")
            nc.sync.dma_start(out=t, in_=bt[n].ap().unsqueeze(0).broadcast_to([H, DIM]))
            bbc[n] = t
        gcol, bcol = {}, {}
        for n in LN_COL:
            t = const.tile([H, 2], F32, tag=f"gc_{n}")
            nc.sync.dma_start(out=t, in_=lng[n].rearrange("(h p) -> p h", p=H))
            gcol[n] = t
            t2 = const.tile([H, 2], F32, tag=f"bc2_{n}")
            nc.sync.dma_start(out=t2, in_=lnb[n].rearrange("(h p) -> p h", p=H))
            bcol[n] = t2
        gbc, bbcn = {}, {}
        for n in LN_BC:
            t = const.tile([H, DIM], F32, tag=f"gbc_{n}")
            nc.sync.dma_start(out=t, in_=lng[n].ap().unsqueeze(0).broadcast_to([H, DIM]))
            gbc[n] = t
            t2 = const.tile([H, DIM], F32, tag=f"bbcn_{n}")
            nc.sync.dma_start(out=t2, in_=lnb[n].ap().unsqueeze(0).broadcast_to([H, DIM]))
            bbcn[n] = t2

        ident32 = const.tile([H, H], F32)
        make_identity(nc, ident32)
        identr = const.tile([H, H], F32R)
        nc.vector.tensor_copy(out=identr, in_=ident32)
        ones_r = const.tile([H, 1], F32)
        nc.vector.memset(ones_r, 1.0)
        epsc = const.tile([H, 1], F32)
        nc.vector.memset(epsc, EPS)

        # helper: layernorm stats -> (rstd, negmurstd) columns, batched.
        # nat_aps: list of [128, 256] APs; returns list of ([128,1], [128,1])
        def ln_stats(nat_aps, tag):
            k = len(nat_aps)
            mv = work3.tile([H, k, 2], F32, tag=f"mv_{tag}")
            for i, ap in enumerate(nat_aps):
                st = work3.tile([H, 6], F32, tag=f"st_{tag}")
                nc.vector.bn_stats(out=st, in_=ap)
                nc.vector.bn_aggr(out=mv[:, i, :], in_=st)
            # rstd = exp(-0.5 * ln(var + eps))
            lv = work3.tile([H, k], F32, tag=f"lv_{tag}")
            nc.scalar.activation(out=lv, in_=mv[:, :, 1], func=AF.Ln, bias=epsc)
            rstd = work3.tile([H, k], F32, tag=f"rs_{tag}")
            nc.scalar.activation(out=rstd, in_=lv, func=AF.Exp, scale=-0.5)
            # negmurstd = -(mean * rstd)
            nmr = work3.tile([H, k], F32, tag=f"nm_{tag}")
            nc.vector.tensor_tensor(out=nmr, in0=mv[:, :, 0], in1=rstd, op=OP.mult)
            nc.scalar.activation(out=nmr, in_=nmr, func=AF.Copy, scale=-1.0)
            return [(rstd[:, i : i + 1], nmr[:, i : i + 1]) for i in range(k)]

        # ================= protein path =================
        px_nat = glob.tile([N, DIM], F32)
        nc.sync.dma_start(out=px_nat, in_=pe_in[:, :])
        ((rs, nm),) = ln_stats([px_nat], "px1")
        pxh = glob.tile([N, DIM], F32)
        nc.scalar.activation(out=pxh, in_=px_nat, func=AF.Identity, scale=rs, bias=nm)
        px_ln = glob.tile([N, DIM], F32R)
        t0 = glob.tile([N, DIM], F32, tag="pxtmp")
        nc.vector.tensor_tensor(out=t0, in0=pxh, in1=gbc["ln1_px"], op=OP.mult)
        nc.vector.tensor_tensor(out=px_ln, in0=t0, in1=bbcn["ln1_px"], op=OP.add)

        # transpose px_ln -> [c2][128, 128]
        def transpose_to(dst, src_nat, n_m1, psum_pool, tagp):
            """src_nat: list of [128,256] f32r nat tiles; dst: 2 tiles
            [128, n_m1*128] (c-half major). Emits PE transposes + ACT copies.
            Optionally scale/bias columns (per c-half) applied on the copy."""
            for h in range(2):
                pt = psum_pool.tile([H, n_m1 * H], F32R, tag="pt")
                for i, s in enumerate(src_nat):
                    nc.tensor.transpose(
                        pt[:, i * H : (i + 1) * H], s[:, h * H : (h + 1) * H], identr)
                yield h, pt

        px_lnT = glob.tile([H, 2, H], F32R)
        for h, pt in transpose_to(None, [px_ln], 1, ps_t, "px"):
            nc.scalar.activation(out=px_lnT[:, h, :], in_=pt, func=AF.Copy)

        # q/k projections (transposed orientation), scale folded into p_q on host
        def proj_T(wn, rhs_halves, n_cols, bias_col, out_dtype=F32):
            """out = W^T @ rhs + bias: returns tile [128, 2, n_cols]"""
            out = glob.tile([H, 2, n_cols], out_dtype, tag=f"projT_{wn}_{n_cols}")
            for m in range(2):
                pm = ps_mm.tile([H, n_cols], F32, tag="mm")
                for k in range(2):
                    nc.tensor.matmul(pm, w[wn][:, k, m, :], rhs_halves[k],
                                     start=(k == 0), stop=(k == 1))
                nc.scalar.activation(out=out[:, m, :], in_=pm, func=AF.Identity,
                                     scale=1.0, bias=bc[wn][:, m : m + 1])
            return out

        qTp = proj_T("p_q", [px_lnT[:, 0, :], px_lnT[:, 1, :]], N, bc["p_q"])
        kTp = proj_T("p_k", [px_lnT[:, 0, :], px_lnT[:, 1, :]], N, bc["p_k"])
        vTp = proj_T("p_v", [px_lnT[:, 0, :], px_lnT[:, 1, :]], N, bc["p_v"])

        # elementwise attention, transposed layout, one n1 at a time:
        # T(n1)[c, n2] = kT[c, n2] * qT[c, n1];  e = exp;  Z/o accumulate.
        zp = glob.tile([H, 2, N], F32, tag="zp")
        ot = glob.tile([H, 2, N], F32, tag="ot")
        for n1 in range(N):
            for h in range(2):
                t = work3.tile([H, N], F32, tag="pT")
                nc.vector.tensor_scalar_mul(
                    out=t, in0=kTp[:, h, :], scalar1=qTp[:, h, n1 : n1 + 1])
                e = work3.tile([H, N], F32, tag="pE")
                nc.scalar.activation(out=e, in_=t, func=AF.Exp,
                                     accum_out=zp[:, h, n1 : n1 + 1])
                ev = work3.tile([H, N], F32, tag="pV")
                nc.vector.scalar_tensor_tensor(
                    out=ev, in0=e, scalar=1.0, in1=vTp[:, h, :],
                    op0=OP.mult, op1=OP.mult,
                    accum_out=ot[:, h, n1 : n1 + 1])
        rzp = glob.tile([H, 2, N], F32, tag="rzp")
        nc.vector.reciprocal(out=rzp, in_=zp)
        oT = glob.tile([H, 2, N], F32R, tag="oT")
        nc.vector.tensor_tensor(out=oT, in0=ot, in1=rzp, op=OP.mult)

        # px1 = px_ln + o @ p_out + b ; ln2
        pp = ps_mm.tile([N, DIM], F32, tag="mm")
        for k in range(2):
            nc.tensor.matmul(pp, oT[:, k, :], w["p_out"][:, k, :],
                             start=(k == 0), stop=(k == 1))
        px1 = glob.tile([N, DIM], F32, tag="px1")
        nc.vector.tensor_tensor(out=px1, in0=pp, in1=bbc["p_out"], op=OP.add)
        nc.vector.tensor_tensor(out=px1, in0=px1, in1=px_ln, op=OP.add)
        ((rs, nm),) = ln_stats([px1], "px2")
        nc.scalar.activation(out=px1, in_=px1, func=AF.Identity, scale=rs, bias=nm)
        px2 = glob.tile([N, DIM], F32R, tag="px2")
        nc.vector.tensor_tensor(out=t0, in0=px1, in1=gbc["ln2_px"], op=OP.mult)
        nc.vector.tensor_tensor(out=px2, in0=t0, in1=bbcn["ln2_px"], op=OP.add)
        px2T = glob.tile([H, 2, N], F32R)
        for h, pt in transpose_to(None, [px2], 1, ps_t, "px2"):
            nc.scalar.activation(out=px2T[:, h, :], in_=pt, func=AF.Copy)

        # enc-dec k: kTe[h] = [128, 128] fp32
        kTe = proj_T("k_px", [px2T[:, 0, :], px2T[:, 1, :]], N, bc["k_px"])

        # ================= mx prolog =================
        R = B_PER_CORE * M  # 256 rows
        mxn = [glob.tile([M, DIM], F32, tag=f"mxn{r}", name=f"mxn{r}") for r in range(B_PER_CORE)]
        for r in range(B_PER_CORE):
            nc.sync.dma_start(out=mxn[r], in_=mx_in[r, :, :])
        stats = ln_stats(mxn, "mx1")
        mxl = []
        for r in range(B_PER_CORE):
            rs, nm = stats[r]
            xh = glob.tile([M, DIM], F32, tag=f"mxh{r}")
            nc.scalar.activation(out=xh, in_=mxn[r], func=AF.Identity, scale=rs, bias=nm)
            ml = glob.tile([M, DIM], F32R, tag=f"mxl{r}")
            tt = work3.tile([M, DIM], F32, tag="mxtmp")
            nc.vector.tensor_tensor(out=tt, in0=xh, in1=gbc["ln1_mx"], op=OP.mult)
            nc.vector.tensor_tensor(out=ml, in0=tt, in1=bbcn["ln1_mx"], op=OP.add)
            mxl.append(ml)
        mx_lnT = glob.tile([H, 2, R], F32R)
        for h, pt in transpose_to(None, mxl, B_PER_CORE, ps_t, "mx"):
            nc.scalar.activation(out=mx_lnT[:, h, :], in_=pt, func=AF.Copy)
        qT = proj_T("q_mx", [mx_lnT[:, 0, :], mx_lnT[:, 1, :]], R, bc["q_mx"])
        vT = proj_T("v_mx", [mx_lnT[:, 0, :], mx_lnT[:, 1, :]], R, bc["v_mx"])

        # ================= main ma loop =================
        ST = 4  # m1 rows per supertile
        W_ = ST * H  # 512
        mx1T = glob.tile([H, 2, R], F32R, tag="mx1T")

        for b in range(B_PER_CORE):
            zb = glob.tile([H, 2, M], F32, tag=f"zb{b}")
            vob = glob.tile([H, 2, M], F32, tag=f"vob{b}")
            for st in range(M // ST):
                m0 = st * ST
                nat = []
                for i in range(ST):
                    t = work.tile([M, DIM], F32, tag=f"nat{i}")
                    nc.sync.dma_start(out=t, in_=ma_in[b, m0 + i, :, :])
                    nat.append(t)
                # LN1 (stats natural), normalize -> f32r
                stats = ln_stats(nat, "ln1")
                xh = []
                for i in range(ST):
                    rs, nm = stats[i]
                    t = work.tile([M, DIM], F32R, tag=f"xh{i}")
                    nc.scalar.activation(out=t, in_=nat[i], func=AF.Identity,
                                         scale=rs, bias=nm)
                    xh.append(t)
                # transpose + gamma/beta -> maT_ln [c2][128, 512]
                maT = [work.tile([H, W_], F32R, tag=f"maT{h}", name=f"maT{h}") for h in range(2)]
                for h, pt in transpose_to(None, xh, ST, ps_t, "ln1"):
                    nc.scalar.activation(
                        out=maT[h], in_=pt, func=AF.Identity,
                        scale=gcol["ln1_ma"][:, h : h + 1],
                        bias=bcol["ln1_ma"][:, h : h + 1])
                # mol_e = maT @ v_ma + b   (psum)
                mol_ps = []
                for m in range(2):
                    pm = ps_mm.tile([H, W_], F32, tag="mm")
                    for k in range(2):
                        nc.tensor.matmul(pm, w["v_ma"][:, k, m, :], maT[k],
                                         start=(k == 0), stop=(k == 1))
                    mol_ps.append(pm)
                # attnT = ((mol_e + b) * k) * q   -> f32r
                attnT = [work.tile([H, W_], F32R, tag=f"at{h}", name=f"at{h}") for h in range(2)]
                for h in range(2):
                    tq = work.tile([H, W_], F32, tag=f"tq{h}")
                    nc.vector.scalar_tensor_tensor(
                        out=tq.rearrange("p (i n) -> p i n", n=H),
                        in0=mol_ps[h].rearrange("p (i n) -> p i n", n=H),
                        scalar=bc["v_ma"][:, h : h + 1],
                        in1=kTe[:, h, :].unsqueeze(1).broadcast_to([H, ST, H]),
                        op0=OP.add, op1=OP.mult)
                    for i in range(ST):
                        nc.vector.tensor_scalar_mul(
                            out=attnT[h][:, i * H : (i + 1) * H],
                            in0=tq[:, i * H : (i + 1) * H],
                            scalar1=qT[:, h, b * M + m0 + i : b * M + m0 + i + 1])
                # ma1T = attnT @ out_ed + b + maT_ln
                ma1T = [work.tile([H, W_], F32R, tag=f"m1T{h}", name=f"m1T{h}") for h in range(2)]
                for m in range(2):
                    pm = ps_mm.tile([H, W_], F32, tag="mm")
                    for k in range(2):
                        nc.tensor.matmul(pm, w["out_ed"][:, k, m, :], attnT[k],
                                         start=(k == 0), stop=(k == 1))
                    nc.vector.scalar_tensor_tensor(
                        out=ma1T[m], in0=pm, scalar=bc["out_ed"][:, m : m + 1],
                        in1=maT[m], op0=OP.add, op1=OP.add)
                # softmax pieces: e = exp(attnT), Z = rowsum; ev-sum via stt
                for h in range(2):
                    e = work.tile([H, W_], F32, tag=f"e{h}")
                    for i in range(ST):
                        nc.scalar.activation(
                            out=e[:, i * H : (i + 1) * H],
                            in_=attnT[h][:, i * H : (i + 1) * H], func=AF.Exp,
                            accum_out=zb[:, h, m0 + i : m0 + i + 1])
                    for i in range(ST):
                        nc.vector.scalar_tensor_tensor(
                            out=e[:, i * H : (i + 1) * H],
                            in0=e[:, i * H : (i + 1) * H], scalar=1.0,
                            in1=vT[:, h, b * M : (b + 1) * M],
                            op0=OP.mult, op1=OP.mult,
                            accum_out=vob[:, h, m0 + i : m0 + i + 1])
                # LN3: transpose back to natural (psum), stats, norm, re-transpose
                pn = []
                for j in range(ST // 2):
                    p = ps_nat.tile([M, 2 * DIM], F32R, tag="nat")
                    pn.append(p)
                for i in range(ST):
                    for h in range(2):
                        nc.tensor.transpose(
                            pn[i // 2][:, (i % 2) * DIM + h * H :
                                       (i % 2) * DIM + (h + 1) * H],
                            ma1T[h][:, i * H : (i + 1) * H], identr)
                nat3 = [pn[i // 2][:, (i % 2) * DIM : (i % 2 + 1) * DIM]
                        for i in range(ST)]
                stats = ln_stats(nat3, "ln3")
                xh3 = []
                for i in range(ST):
                    rs, nm = stats[i]
                    t = work.tile([M, DIM], F32R, tag=f"x3{i}")
                    nc.scalar.activation(out=t, in_=nat3[i], func=AF.Identity,
                                         scale=rs, bias=nm)
                    xh3.append(t)
                ma2T = [work.tile([H, W_], F32R, tag=f"m2T{h}", name=f"m2T{h}") for h in range(2)]
                for h, pt in transpose_to(None, xh3, ST, ps_t, "ln3"):
                    nc.scalar.activation(
                        out=ma2T[h], in_=pt, func=AF.Identity,
                        scale=gcol["ln3_ma"][:, h : h + 1],
                        bias=bcol["ln3_ma"][:, h : h + 1])
                # mlp: fc1 relu, fc2, + ma2T residual
                hT = [work.tile([H, W_], F32R, tag=f"hT{h}", name=f"hT{h}") for h in range(2)]
                for m in range(2):
                    pm = ps_mm.tile([H, W_], F32, tag="mm")
                    for k in range(2):
                        nc.tensor.matmul(pm, w["ma_fc1"][:, k, m, :], ma2T[k],
                                         start=(k == 0), stop=(k == 1))
                    nc.scalar.activation(out=hT[m], in_=pm, func=AF.Relu,
                                         bias=bc["ma_fc1"][:, m : m + 1])
                yT = [work.tile([H, W_], F32R, tag=f"yT{h}", name=f"yT{h}") for h in range(2)]
                for m in range(2):
                    pm = ps_mm.tile([H, W_], F32, tag="mm")
                    for k in range(2):
                        nc.tensor.matmul(pm, w["ma_fc2"][:, k, m, :], hT[k],
                                         start=(k == 0), stop=(k == 1))
                    nc.vector.scalar_tensor_tensor(
                        out=yT[m], in0=pm, scalar=bc["ma_fc2"][:, m : m + 1],
                        in1=ma2T[m], op0=OP.add, op1=OP.add)
                # LN4 -> natural out + gamma/beta + store
                pn4 = []
                for j in range(ST // 2):
                    p = ps_nat.tile([M, 2 * DIM], F32R, tag="nat")
                    pn4.append(p)
                for i in range(ST):
                    for h in range(2):
                        nc.tensor.transpose(
                            pn4[i // 2][:, (i % 2) * DIM + h * H :
                                        (i % 2) * DIM + (h + 1) * H],
                            yT[h][:, i * H : (i + 1) * H], identr)
                nat4 = [pn4[i // 2][:, (i % 2) * DIM : (i % 2 + 1) * DIM]
                        for i in range(ST)]
                stats = ln_stats(nat4, "ln4")
                for i in range(ST):
                    rs, nm = stats[i]
                    x4 = work.tile([M, DIM], F32, tag=f"x4{i}")
                    nc.scalar.activation(out=x4, in_=nat4[i], func=AF.Identity,
                                         scale=rs, bias=nm)
                    nc.vector.tensor_tensor(out=x4, in0=x4, in1=gbc["ln4_ma"],
                                            op=OP.mult)
                    nc.gpsimd.tensor_tensor(out=x4, in0=x4, in1=bbcn["ln4_ma"],
                                            op=OP.add)
                    nc.sync.dma_start(out=ma_out[b, m0 + i, :, :], in_=x4)

            # ---- per-batch mx update: vo = (sum ev) / Z, mx1 = mx_ln + vo@W+b
            rzb = work.tile([H, 2, M], F32, tag="rzb")
            nc.vector.reciprocal(out=rzb, in_=zb)
            von = work.tile([H, 2, M], F32R, tag="von")
            nc.vector.tensor_tensor(out=von, in0=vob, in1=rzb, op=OP.mult)
            for m in range(2):
                pm = ps_mm.tile([H, M], F32, tag="mm")
                for k in range(2):
                    nc.tensor.matmul(pm, w["out_nd"][:, k, m, :], von[:, k, :],
                                     start=(k == 0), stop=(k == 1))
                nc.vector.scalar_tensor_tensor(
                    out=mx1T[:, m, b * M : (b + 1) * M], in0=pm,
                    scalar=bc["out_nd"][:, m : m + 1],
                    in1=mx_lnT[:, m, b * M : (b + 1) * M],
                    op0=OP.add, op1=OP.add)

        # ================= mx tail =================
        # LN3_mx
        pnm = [ps_nat.tile([M, 2 * DIM], F32R, tag="nat", name=f"pnm{r}")
               for r in range(B_PER_CORE // 2 + (B_PER_CORE % 2))]
        for r in range(B_PER_CORE):
            for h in range(2):
                nc.tensor.transpose(
                    pnm[r // 2][:, (r % 2) * DIM + h * H : (r % 2) * DIM + (h + 1) * H],
                    mx1T[:, h, r * M : (r + 1) * M], identr)
        natm = [pnm[r // 2][:, (r % 2) * DIM : (r % 2 + 1) * DIM]
                for r in range(B_PER_CORE)]
        stats = ln_stats(natm, "ln3mx")
        xh3m = []
        for r in range(B_PER_CORE):
            rs, nm = stats[r]
            t = glob.tile([M, DIM], F32R, tag=f"x3m{r}")
            nc.scalar.activation(out=t, in_=natm[r], func=AF.Identity,
                                 scale=rs, bias=nm)
            xh3m.append(t)
        mx2T = glob.tile([H, 2, R], F32R, tag="mx2T")
        for h, pt in transpose_to(None, xh3m, B_PER_CORE, ps_t, "ln3mx"):
            nc.scalar.activation(
                out=mx2T[:, h, :], in_=pt, func=AF.Identity,
                scale=gcol["ln3_mx"][:, h : h + 1],
                bias=bcol["ln3_mx"][:, h : h + 1])
        hTm = glob.tile([H, 2, R], F32R, tag="hTm")
        for m in range(2):
            pm = ps_mm.tile([H, R], F32, tag="mm")
            for k in range(2):
                nc.tensor.matmul(pm, w["mx_fc1"][:, k, m, :], mx2T[:, k, :],
                                 start=(k == 0), stop=(k == 1))
            nc.scalar.activation(out=hTm[:, m, :], in_=pm, func=AF.Relu,
                                 bias=bc["mx_fc1"][:, m : m + 1])
        yTm = glob.tile([H, 2, R], F32R, tag="yTm")
        for m in range(2):
            pm = ps_mm.tile([H, R], F32, tag="mm")
            for k in range(2):
                nc.tensor.matmul(pm, w["mx_fc2"][:, k, m, :], hTm[:, k, :],
                                 start=(k == 0), stop=(k == 1))
            nc.vector.scalar_tensor_tensor(
                out=yTm[:, m, :], in0=pm, scalar=bc["mx_fc2"][:, m : m + 1],
                in1=mx2T[:, m, :], op0=OP.add, op1=OP.add)
        # LN4_mx -> natural + g/b + store
        pnm4 = [ps_nat.tile([M, 2 * DIM], F32R, tag="nat", name=f"pnm4{r}")
                for r in range(B_PER_CORE // 2 + (B_PER_CORE % 2))]
        for r in range(B_PER_CORE):
            for h in range(2):
                nc.tensor.transpose(
                    pnm4[r // 2][:, (r % 2) * DIM + h * H : (r % 2) * DIM + (h + 1) * H],
                    yTm[:, h, r * M : (r + 1) * M], identr)
        natm4 = [pnm4[r // 2][:, (r % 2) * DIM : (r % 2 + 1) * DIM]
                 for r in range(B_PER_CORE)]
        stats = ln_stats(natm4, "ln4mx")
        for r in range(B_PER_CORE):
            rs, nm = stats[r]
            x4 = glob.tile([M, DIM], F32, tag=f"x4m{r}")
            nc.scalar.activation(out=x4, in_=natm4[r], func=AF.Identity,
                                 scale=rs, bias=nm)
            nc.vector.tensor_tensor(out=x4, in0=x4, in1=gbc["ln4_mx"], op=OP.mult)
            nc.gpsimd.tensor_tensor(out=x4, in0=x4, in1=bbcn["ln4_mx"], op=OP.add)
            nc.sync.dma_start(out=mx_out[r, :, :], in_=x4)


# ---------------------------------------------------------------------------
# host glue
# ---------------------------------------------------------------------------

_PROG = None


def _get_program():
    global _PROG
    if _PROG is None:
        _PROG = build_program()
    return _PROG


def _np(x):
    return np.asarray(x, dtype=np.float32)


def kernel(mol_annot, mol_adj, protein_embedding, params):
    from concourse.bass_utils import run_bass_kernel_spmd

    nc = _get_program()
    scale = 1.0 / np.sqrt(np.float32(DIM))

    p = params
    wmap = {
        "v_ma": p["v_ma"], "out_ed": p["out_ed"], "out_nd": p["out_nd"],
        "q_mx": p["q_mx"], "v_mx": p["v_mx"], "k_px": p["k_px"],
        "p_q": p["p_q"], "p_k": p["p_k"], "p_v": p["p_v"], "p_out": p["p_out"],
        "ma_fc1": p["mlp_ma"]["fc1"], "ma_fc2": p["mlp_ma"]["fc2"],
        "mx_fc1": p["mlp_mx"]["fc1"], "mx_fc2": p["mlp_mx"]["fc2"],
    }
    common = {}
    for n, d in wmap.items():
        wv, bv = _np(d["w"]), _np(d["b"])
        if n in ("q_mx", "p_q"):  # fold the attention scale into q
            wv, bv = wv * scale, bv * scale
        common[f"w_{n}"] = np.ascontiguousarray(wv)
        common[f"b_{n}"] = np.ascontiguousarray(bv)
    lnmap = {
        "ln1_ma": p["ln1_ma"], "ln3_ma": p["ln3_ma"], "ln3_mx": p["ln3_mx"],
        "ln1_mx": p["ln1_mx"], "ln1_px": p["ln1_px"], "ln2_px": p["ln2_px"],
        "ln4_ma": p["ln4_ma"], "ln4_mx": p["ln4_mx"],
    }
    for n, d in lnmap.items():
        common[f"g_{n}"] = _np(d["g"])
        common[f"bb_{n}"] = _np(d["b"])
    common["pe_in"] = _np(protein_embedding)

    ma = _np(mol_adj)
    mx = _np(mol_annot)
    in_maps = []
    for c in range(N_CORES):
        m = dict(common)
        m["ma_in"] = np.ascontiguousarray(ma[c * B_PER_CORE : (c + 1) * B_PER_CORE])
        m["mx_in"] = np.ascontiguousarray(mx[c * B_PER_CORE : (c + 1) * B_PER_CORE])
        in_maps.append(m)

    res = run_bass_kernel_spmd(nc, in_maps, core_ids=list(range(N_CORES)))
    mx_out = np.concatenate([res.results[c]["mx_out"] for c in range(N_CORES)], 0)
    ma_out = np.concatenate([res.results[c]["ma_out"] for c in range(N_CORES)], 0)
    return mx_out, ma_out


if __name__ == "__main__":
    nc = build_program()
    n = sum(len(bb.instructions) for f in nc.m.functions for bb in f.blocks)
    print("instructions:", n)
